# revision 14
# baseline (speedup 1.0000x reference)
"""FFT-block kernel for Trainium2 (8 NeuronCores, batch-data-parallel).

Computation (per sample):
  y0  = mean(x, (H, W))                      [C]
  h   = relu(y0 @ W1c.T + b1)                [C/6]
  y   = sigmoid(h @ W2c.T + b2)              [C]
  s1  = relu(y @ Ws1.T + bs1)                [CF]
  s2  = relu(y @ Ws2.T + bs2)                [CF]
  yf  = rfft(y); amp=|yf|*s1; pha=angle(yf)*s2
  rec = amp*(cos(pha) + i sin(pha)); xr = irfft(rec, C)
  out = (xr * y)[:, None, None]

Strategy: batch dim (16) sharded 2-per-core. The 400MB stream of x feeds a
free-axis reduction (DMA-bound); the tiny MLP/FFT tail runs on-chip in a
"transposed" layout (channels/freq bins on partitions, batch on the free
axis). rfft/irfft are matmuls against host-precomputed DFT basis matrices.
angle() uses atan2(y,x) = 2*atan(y/(|z|+x)); the DC and Nyquist bins (where
Im==0 analytically) are special-cased via sign(Re) to avoid noise-driven
phase errors. cos(t) = sin(t + pi/2).
"""

import numpy as np
from contextlib import ExitStack

import concourse.bass as bass
import concourse.bacc as bacc
import concourse.tile as tile
from concourse import mybir
from concourse.bass_utils import run_bass_kernel_spmd

B, C, H, W = 16, 384, 128, 128
NCORES = 8
BPC = B // NCORES            # 2 samples per core
CH = C // 6                  # 64
CF = C // 2 + 1              # 193 rfft bins
HW = H * W                   # 16384
FP32 = mybir.dt.float32
AF = mybir.ActivationFunctionType
AX = mybir.AxisListType

F_CHUNK = 4096               # free-dim chunk of the x stream
N_CHUNK = HW // F_CHUNK      # 4
STREAM_BUFS = 8

KC = [0, 128, 256]           # channel chunks (3 x 128)
FC = [(0, 128), (128, 65)]   # freq-bin chunks (128 + 65)
NYQ = 192                    # Nyquist bin index


def _build():
    nc = bacc.Bacc(
        "TRN2",
        target_bir_lowering=False,
        debug=False,
        enable_asserts=False,
        num_devices=NCORES,
    )

    xs = nc.dram_tensor("xs", [BPC, C, H, W], FP32, kind="ExternalInput")
    w1t = nc.dram_tensor("w1t", [C, CH], FP32, kind="ExternalInput")
    b1c = nc.dram_tensor("b1c", [CH, 1], FP32, kind="ExternalInput")
    w2t = nc.dram_tensor("w2t", [CH, C], FP32, kind="ExternalInput")
    b2c = nc.dram_tensor("b2c", [C, 1], FP32, kind="ExternalInput")
    ws1t = nc.dram_tensor("ws1t", [C, CF], FP32, kind="ExternalInput")
    bs1c = nc.dram_tensor("bs1c", [CF, 1], FP32, kind="ExternalInput")
    ws2t = nc.dram_tensor("ws2t", [C, CF], FP32, kind="ExternalInput")
    bs2c = nc.dram_tensor("bs2c", [CF, 1], FP32, kind="ExternalInput")
    cmat = nc.dram_tensor("cmat", [C, CF], FP32, kind="ExternalInput")
    smat = nc.dram_tensor("smat", [C, CF], FP32, kind="ExternalInput")
    icrm = nc.dram_tensor("icrm", [CF, C], FP32, kind="ExternalInput")
    icim = nc.dram_tensor("icim", [CF, C], FP32, kind="ExternalInput")
    outp = nc.dram_tensor("out", [BPC, C, 1, 1], FP32, kind="ExternalOutput")

    with tile.TileContext(nc) as tc, ExitStack() as ctx:
        persist = ctx.enter_context(tc.tile_pool(name="persist", bufs=1))
        stream = ctx.enter_context(tc.tile_pool(name="stream", bufs=STREAM_BUFS))
        psum = ctx.enter_context(
            tc.tile_pool(name="psum", bufs=8, space=bass.MemorySpace.PSUM)
        )

        def load_const(tag, dram_ap, shape):
            t = persist.tile(shape, FP32, tag=tag, name=tag)
            nc.scalar.dma_start(out=t, in_=dram_ap)
            return t

        # ---- small constants (loaded once, overlap with the x stream) ----
        w1t_sb = [load_const(f"w1t{k}", w1t[s : s + 128, :], [128, CH]) for k, s in enumerate(KC)]
        w2t_sb = load_const("w2t", w2t[:, :], [CH, C])
        b1_sb = load_const("b1", b1c[:, :], [CH, 1])
        b2_sb = [load_const(f"b2_{m}", b2c[s : s + 128, :], [128, 1]) for m, s in enumerate(KC)]
        proj_mats = {}
        for nm, dr in (("ws1t", ws1t), ("ws2t", ws2t), ("cmat", cmat), ("smat", smat)):
            proj_mats[nm] = [
                load_const(f"{nm}{k}", dr[s : s + 128, :], [128, CF]) for k, s in enumerate(KC)
            ]
        bs1_sb = [load_const(f"bs1{j}", bs1c[s : s + l, :], [l, 1]) for j, (s, l) in enumerate(FC)]
        bs2_sb = [load_const(f"bs2{j}", bs2c[s : s + l, :], [l, 1]) for j, (s, l) in enumerate(FC)]
        icr_sb = [load_const(f"icr{j}", icrm[s : s + l, :], [l, C]) for j, (s, l) in enumerate(FC)]
        ici_sb = [load_const(f"ici{j}", icim[s : s + l, :], [l, C]) for j, (s, l) in enumerate(FC)]

        # bias constants for the activation unit (float biases need SBUF APs)
        pio2 = persist.tile([128, 1], FP32, tag="pio2", name="pio2")
        nc.vector.memset(pio2, np.pi / 2)
        pio4 = persist.tile([128, 1], FP32, tag="pio4", name="pio4")
        nc.vector.memset(pio4, np.pi / 4)
        epsc = persist.tile([128, 1], FP32, tag="epsc", name="epsc")
        nc.vector.memset(epsc, 1e-30)

        # ---- phase 1: stream x, per-(b,c)-row sums over the spatial axis ----
        # rows of [BPC*C, HW]; row-tile t holds channels of (b = t//3, kchunk = t%3).
        # Chunks alternate between DVE (reduce) and ACT (Identity + accum_out
        # row-sum side output) so neither engine alone paces the DMA stream.
        xrows = xs.rearrange("b c h w -> (b c) (h w)")
        dummy = persist.tile([128, F_CHUNK], mybir.dt.bfloat16, tag="dummy", name="dummy")
        yt = [persist.tile([128, BPC], FP32, tag=f"yt{k}", name=f"yt{k}") for k in range(3)]
        for t in range(BPC * 3):
            b, k = divmod(t, 3)
            # one partial tile per chunk: shared tiles would create false
            # WAW deps in Tile's tracker and serialize the two reduce engines
            parts = [
                persist.tile([128, 1], FP32, tag=f"part{t}_{j}", name=f"part{t}_{j}")
                for j in range(N_CHUNK)
            ]
            for j in range(N_CHUNK):
                chk = stream.tile([128, F_CHUNK], FP32, tag="stream", name=f"chk{t}_{j}")
                nc.sync.dma_start(
                    out=chk,
                    in_=xrows[t * 128 : (t + 1) * 128, j * F_CHUNK : (j + 1) * F_CHUNK],
                )
                if j % 2 == 0:
                    nc.vector.reduce_sum(out=parts[j], in_=chk, axis=AX.X)
                else:
                    nc.scalar.activation(
                        out=dummy[:, :], in_=chk, func=AF.Identity,
                        accum_out=parts[j],
                    )
            a01 = persist.tile([128, 1], FP32, tag=f"a01_{t}", name=f"a01_{t}")
            nc.vector.tensor_add(out=a01, in0=parts[0], in1=parts[1])
            a23 = persist.tile([128, 1], FP32, tag=f"a23_{t}", name=f"a23_{t}")
            nc.vector.tensor_add(out=a23, in0=parts[2], in1=parts[3])
            nc.vector.tensor_add(out=yt[k][:, b : b + 1], in0=a01, in1=a23)
        # yt holds raw sums; the 1/HW mean scale is folded into w1t host-side.

        # ---- phase 2: MLP tail, transposed layout [chan/freq (part), batch] ----
        # h = relu(W1c^T y0 + b1)            [CH, BPC]
        ph = psum.tile([CH, BPC], FP32, tag="mm", name="ph")
        for k in range(3):
            nc.tensor.matmul(ph, lhsT=w1t_sb[k], rhs=yt[k], start=(k == 0), stop=(k == 2))
        h_sb = persist.tile([CH, BPC], FP32, tag="h", name="h_sb")
        nc.scalar.activation(out=h_sb, in_=ph, func=AF.Relu, bias=b1_sb)

        # y = sigmoid(W2c^T h + b2)          3 x [128, BPC]
        y_sb = []
        for m, s in enumerate(KC):
            py = psum.tile([128, BPC], FP32, tag="mm", name=f"py{m}")
            nc.tensor.matmul(py, lhsT=w2t_sb[:, s : s + 128], rhs=h_sb, start=True, stop=True)
            yb = persist.tile([128, BPC], FP32, tag=f"y{m}", name=f"y{m}")
            nc.scalar.activation(out=yb, in_=py, func=AF.Sigmoid, bias=b2_sb[m])
            y_sb.append(yb)

        # projections of y through [C, CF] matrices -> [CF, BPC] in 2 chunks
        def proj(mats, biases, func, nm):
            outs = []
            for j, (s, l) in enumerate(FC):
                pt = psum.tile([l, BPC], FP32, tag="mm", name=f"p{nm}{j}")
                for k in range(3):
                    nc.tensor.matmul(
                        pt, lhsT=mats[k][:, s : s + l], rhs=y_sb[k],
                        start=(k == 0), stop=(k == 2),
                    )
                ob = persist.tile([l, BPC], FP32, tag=f"{nm}{j}", name=f"{nm}{j}")
                if func is None:
                    nc.vector.tensor_copy(out=ob, in_=pt)
                else:
                    nc.scalar.activation(out=ob, in_=pt, func=func, bias=biases[j])
                outs.append(ob)
            return outs

        s1 = proj(proj_mats["ws1t"], bs1_sb, AF.Relu, "s1")
        s2 = proj(proj_mats["ws2t"], bs2_sb, AF.Relu, "s2")
        re = proj(proj_mats["cmat"], None, None, "re")
        im = proj(proj_mats["smat"], None, None, "im")

        # rec = amp * exp(i * pha);  amp = |yf|*s1, pha = angle(yf)*s2
        # HW activation-table ranges: Arctan needs |x| <= pi/2, Sin |x| <= pi.
        # atan via u = min(|t|, 1/|t|) in [0,1], unfolded with sign/compare;
        # sin/cos via exact mod-2pi reduction (fp32 round-to-int magic trick).
        EPS = 1e-30
        MAGIC = 12582912.0  # 1.5 * 2**23: x+MAGIC-MAGIC == round(x) in fp32
        recre, recim = [], []
        for j, (s, l) in enumerate(FC):
            def T(tg):
                return persist.tile([l, BPC], FP32, tag=f"{tg}{j}", name=f"{tg}{j}")

            r2 = T("r2")
            nc.vector.tensor_mul(out=r2, in0=re[j], in1=re[j])
            i2 = T("i2")
            nc.vector.tensor_mul(out=i2, in0=im[j], in1=im[j])
            nc.vector.tensor_add(out=r2, in0=r2, in1=i2)
            amp0 = T("amp0")
            nc.scalar.activation(out=amp0, in_=r2, func=AF.Sqrt)
            den = T("den")  # |z| + Re, kept strictly positive
            nc.vector.scalar_tensor_tensor(
                out=den, in0=amp0, scalar=EPS, in1=re[j],
                op0=mybir.AluOpType.add, op1=mybir.AluOpType.add,
            )
            nc.vector.reciprocal(out=den, in_=den)
            tq = T("tq")  # t = Im / (|z| + Re) = tan(angle/2)
            nc.vector.tensor_mul(out=tq, in0=im[j], in1=den)
            ab = T("ab")
            nc.scalar.activation(out=ab, in_=tq, func=AF.Abs, bias=epsc[:l, :])
            inv = T("inv")
            nc.vector.reciprocal(out=inv, in_=ab)
            u = T("u")
            nc.vector.tensor_tensor(out=u, in0=ab, in1=inv, op=mybir.AluOpType.min)
            a = T("a")  # atan(u) in [0, pi/4]
            nc.scalar.activation(out=a, in_=u, func=AF.Arctan)
            w = T("w")  # pi/2 - 2a, applied where |t| > 1
            nc.scalar.activation(
                out=w, in_=a, func=AF.Identity, scale=-2.0, bias=pio2[:l, :]
            )
            fgt = T("fgt")
            nc.vector.tensor_scalar(
                out=fgt, in0=ab, scalar1=1.0, scalar2=None, op0=mybir.AluOpType.is_gt
            )
            nc.vector.tensor_mul(out=w, in0=w, in1=fgt)
            nc.vector.tensor_add(out=a, in0=a, in1=w)  # |angle|/2
            sgt = T("sgt")
            nc.scalar.activation(out=sgt, in_=tq, func=AF.Sign)
            at = T("at")  # angle/2
            nc.vector.tensor_mul(out=at, in0=a, in1=sgt)
            # DC (j0,p0) / Nyquist (j1,p64) bins: Im==0 analytically, so the
            # half-angle quotient is noise-driven there. angle is exactly 0
            # (Re>0) or pi (Re<0): at = pi/4 * (1 - sign(Re)).
            p = 0 if j == 0 else NYQ - FC[1][0]
            sg = persist.tile([1, BPC], FP32, tag=f"sg{j}", name=f"sg{j}")
            nc.scalar.activation(out=sg, in_=re[j][p : p + 1, :], func=AF.Sign)
            nc.scalar.activation(
                out=at[p : p + 1, :], in_=sg, func=AF.Identity,
                scale=-np.pi / 4, bias=pio4[:1, :],
            )
            q = T("q")  # q = (angle/2)*s2; pha = 2q
            nc.vector.tensor_mul(out=q, in0=at, in1=s2[j])
            r = T("r")  # pha / 2pi
            nc.vector.tensor_scalar_mul(out=r, in0=q, scalar1=float(1.0 / np.pi))
            rc = T("rc")  # (pha + pi/2) / 2pi
            nc.vector.tensor_scalar_add(out=rc, in0=r, scalar1=0.25)
            n1 = T("n1")
            nc.vector.tensor_scalar(
                out=n1, in0=r, scalar1=MAGIC, scalar2=MAGIC,
                op0=mybir.AluOpType.add, op1=mybir.AluOpType.subtract,
            )
            nc.vector.tensor_sub(out=n1, in0=r, in1=n1)  # frac(r) in [-.5, .5]
            sn = T("sn")
            nc.scalar.activation(out=sn, in_=n1, func=AF.Sin, scale=float(2 * np.pi))
            n2 = T("n2")
            nc.vector.tensor_scalar(
                out=n2, in0=rc, scalar1=MAGIC, scalar2=MAGIC,
                op0=mybir.AluOpType.add, op1=mybir.AluOpType.subtract,
            )
            nc.vector.tensor_sub(out=n2, in0=rc, in1=n2)
            cs = T("cs")
            nc.scalar.activation(out=cs, in_=n2, func=AF.Sin, scale=float(2 * np.pi))
            amp = T("amp")
            nc.vector.tensor_mul(out=amp, in0=amp0, in1=s1[j])
            rr = T("rr")
            nc.vector.tensor_mul(out=rr, in0=amp, in1=cs)
            ri = T("ri")
            nc.vector.tensor_mul(out=ri, in0=amp, in1=sn)
            recre.append(rr)
            recim.append(ri)

        # xr = irfft(rec) via basis matmuls; out = xr * y -> DRAM
        out_all = persist.tile([128, 3, BPC], FP32, tag="out_all", name="out_all")
        for m, s in enumerate(KC):
            px = psum.tile([128, BPC], FP32, tag="mm", name=f"px{m}")
            steps = [
                (icr_sb[0], recre[0]), (icr_sb[1], recre[1]),
                (ici_sb[0], recim[0]), (ici_sb[1], recim[1]),
            ]
            for idx, (mt, vt) in enumerate(steps):
                nc.tensor.matmul(
                    px, lhsT=mt[:, s : s + 128], rhs=vt,
                    start=(idx == 0), stop=(idx == len(steps) - 1),
                )
            nc.vector.tensor_mul(out=out_all[:, m, :], in0=px, in1=y_sb[m])
            # out[b, m*128+p, 0, 0] <- out_all[p, m, b]
            base = outp.ap()
            dst = bass.AP(tensor=base.tensor, offset=s, ap=[[1, 128], [C, BPC]])
            nc.sync.dma_start(out=dst, in_=out_all[:, m, :])

    nc.compile()
    return nc


_CACHE = {}


def _get_nc():
    if "nc" not in _CACHE:
        _CACHE["nc"] = _build()
    return _CACHE["nc"]


def _host_prep(inputs):
    f32 = np.float32
    W1 = np.asarray(inputs["W1"], f32)
    W2 = np.asarray(inputs["W2"], f32)
    Ws1 = np.asarray(inputs["Ws1"], f32)
    Ws2 = np.asarray(inputs["Ws2"], f32)
    # center taps of the 3x3 convs; fold the 1/HW mean scale into W1
    w1t = np.ascontiguousarray(W1[:, :, 1, 1].T.astype(np.float64) / HW).astype(f32)
    w2t = np.ascontiguousarray(W2[:, :, 1, 1].T)
    ws1t = np.ascontiguousarray(Ws1.T)
    ws2t = np.ascontiguousarray(Ws2.T)

    i = np.arange(C, dtype=np.float64)[:, None]
    k = np.arange(CF, dtype=np.float64)[None, :]
    ang = 2.0 * np.pi * i * k / C
    cmat = np.cos(ang).astype(f32)
    smat = (-np.sin(ang)).astype(f32)

    kk = np.arange(CF, dtype=np.float64)[:, None]
    n = np.arange(C, dtype=np.float64)[None, :]
    ang2 = 2.0 * np.pi * kk * n / C
    alpha = np.full((CF, 1), 2.0)
    alpha[0, 0] = 1.0
    alpha[NYQ, 0] = 1.0
    icrm = (alpha * np.cos(ang2) / C).astype(f32)
    icim = (-alpha * np.sin(ang2) / C).astype(f32)

    return dict(
        w1t=w1t,
        b1c=np.ascontiguousarray(np.asarray(inputs["b1"], f32).reshape(CH, 1)),
        w2t=w2t,
        b2c=np.ascontiguousarray(np.asarray(inputs["b2"], f32).reshape(C, 1)),
        ws1t=ws1t,
        bs1c=np.ascontiguousarray(np.asarray(inputs["bs1"], f32).reshape(CF, 1)),
        ws2t=ws2t,
        bs2c=np.ascontiguousarray(np.asarray(inputs["bs2"], f32).reshape(CF, 1)),
        cmat=cmat,
        smat=smat,
        icrm=icrm,
        icim=icim,
    )


def kernel(**inputs):
    x = np.asarray(inputs["x"], np.float32)
    base = _host_prep(inputs)
    nc = _get_nc()
    in_maps = [
        dict(base, xs=np.ascontiguousarray(x[i * BPC : (i + 1) * BPC]))
        for i in range(NCORES)
    ]
    res = run_bass_kernel_spmd(nc, in_maps, list(range(NCORES))).results
    return np.concatenate([res[i]["out"] for i in range(NCORES)], axis=0)
